# revision 20
# baseline (speedup 1.0000x reference)
"""Trainium2 Bass kernel for nn_MultiHeadSelfAttention_88725434400988.

Self-contained: accepts FULL inputs, shards batch B=256 over 8 NeuronCores
(32 per core), runs one SPMD Bass program, gathers the FULL output.

Per-core schedule (B_CORE=32, S=8, F=32, E=64, A=64, NH=2), v4:
  - fp16 weights/activations, fp32 PSUM accumulation.
  - WEIGHT FOLD: the output projection distributes over the attention sum,
      out[q,e] = relu( sum_g (sum_k attn_g[q,k] * vw_g[k,e]) + bias )
    with vw = (Wv @ Wres-half) precomputed PER (key-parity chi, query-parity
    j) on the host as one (65, 256) matrix; the bias rides row 64 (driven by
    a ones-row appended to hsv, halved since both j-slabs add it) and the
    softmax denominators ride column 64 of the attention rhs (ones).  This
    removes the separate v/residual matmuls, the ut transpose problem, and
    the whole residual epilogue.
  - Head: hsT on sync, hsv halves on scalar, wvres on gpsimd.  PE: clock
    warm burst then the first half of the vw projection (covers the weight
    stream head).
  - QK projection INTERLEAVED chunk-by-chunk (Wq on sync, Wk on scalar,
    ~150GB/s each, both streams continuous), deep prefetch on both;
    psum evac split ScalarE (rows 0:64) / VectorE (rows 64:128);
    partition shifts on gpsimd/sync; vw-projection pairs sprinkled through
    the stream tail as PE filler so a late chunk cannot idle the PE and
    latch the k=4 HAM clock.
  - Attention per batch b: z^T (4 matmuls, keys on partitions), one
    (128,1024) exp on ScalarE -> bf16, then 8 avw passes
    lhsT=exp(z)[keys, q-parity-half] (full M=128), rhs=[vw|1] (N=65):
    slab[q, g, j, 0:64] unnormalized output, col 64 = denominator.
    reciprocal_approx_fast + per-g fused (mul, add) VectorE pair combines
    the j-slabs; relu+fp16 cast on the Pool engine which also streams the
    output DMAs.  ScalarE exp (1.11us/b) is the phase pacer; PE is at
    ~60% so stream jitter never throttles the clock.
"""
import numpy as np

B, S, F, E, A, NH = 256, 8, 32, 64, 64, 2
NCORES = 8
BC = B // NCORES            # 32 batches per core
ROWS = BC * S               # 256 projection rows
CD = F * E                  # 2048 contraction dim
ND = A * F * NH             # 4096 projection cols
KTILES = CD // 128          # 16
TTILES = ND // 128          # 32 column tiles per weight
NB = BC * NH                # 64 attention batches per core
WCHUNK = 2                  # weight tiles per DMA chunk / psum group
NCHUNK = TTILES // WCHUNK   # 16 chunks per weight

_NC_CACHE = None


def build_bass():
    import concourse.bacc as bacc
    import concourse.tile as tile
    from concourse import mybir

    f16 = mybir.dt.float16
    bf16 = mybir.dt.bfloat16
    f32 = mybir.dt.float32
    Exp = mybir.ActivationFunctionType.Exp
    Copy = mybir.ActivationFunctionType.Copy
    Add = mybir.AluOpType.add
    Max = mybir.AluOpType.max
    Mult = mybir.AluOpType.mult

    nc = bacc.Bacc("TRN2", target_bir_lowering=False, debug=False)

    hst_d = nc.dram_tensor("hst", [128, KTILES, ROWS], f16, kind="ExternalInput")
    hsv_d = nc.dram_tensor("hsv", [E, NB, 128], f16, kind="ExternalInput")
    wq_d = nc.dram_tensor("wq", [128, TTILES, KTILES * 128], f16,
                          kind="ExternalInput")
    wk_d = nc.dram_tensor("wk", [128, TTILES, KTILES * 128], f16,
                          kind="ExternalInput")
    wvres_d = nc.dram_tensor("wvres", [E + 1, 256], f16, kind="ExternalInput")
    out_d = nc.dram_tensor("out", [128, BC * 128], f16, kind="ExternalOutput")

    with tile.TileContext(nc) as tc:
        from contextlib import ExitStack
        with ExitStack() as ctx:
            singles = ctx.enter_context(tc.tile_pool(name="singles", bufs=1))

            # ---- constants / persistent tiles ----
            ones_bf = singles.tile([128, A], bf16)
            nc.vector.memset(ones_bf, 1.0)
            warm_t = singles.tile([128, 256], f16)
            nc.vector.memset(warm_t, 0.25)
            dummy_e = singles.tile([128, 8], bf16)

            hsT = singles.tile([128, KTILES, ROWS], f16)
            hsv = singles.tile([E + 1, NB, 128], f16)   # row 64 = ones (bias)
            nc.vector.memset(hsv[E:E + 1, :, :], 1.0)
            wvres_sb = singles.tile([E + 1, 256], f16)

            qt = singles.tile([64, 2, BC, NH, 128], f16)
            kt_ = singles.tile([64, 2, BC, NH, 128], f16)
            # vw[sigma, bn, chi(=kk), j, 0:64]=folded V*Wres rows; col 64=1
            vw_all = singles.tile([128, NB, 2, 2, 65], bf16)
            nc.vector.memset(vw_all[:, :, :, :, 64:65], 1.0)

            # ---- input DMAs: one bulk stream head per queue ----
            nc.sync.dma_start(hsT[:, :, :], hst_d[:])
            nc.scalar.dma_start(hsv[0:E, 0:NB // 2, :], hsv_d[:, 0:NB // 2, :])
            nc.scalar.dma_start(hsv[0:E, NB // 2:NB, :],
                                hsv_d[:, NB // 2:NB, :])
            nc.gpsimd.dma_start(wvres_sb[:, :], wvres_d[:])

            # pre-load the Exp act table during the head
            nc.scalar.activation(dummy_e[:, :], warm_t[:, 0:8], Exp)

            # ---- PE clock warm-up on memset data while the head DMAs run --
            with tc.tile_pool(name="pwarm", bufs=1, space="PSUM") as pw_pool:
                pw = pw_pool.tile([A, 256], f32)
                for wi in range(22):
                    nc.tensor.matmul(
                        pw[:, :], lhsT=ones_bf[:, :], rhs=warm_t[:, :],
                        start=(wi == 0), stop=(wi == 21))

            def emit_vproj(pair, vps_pool):
                vt = vps_pool.tile([128, 2, 256], f32, name="vt", tag="vt")
                for jb in range(2):
                    nc.tensor.matmul(
                        vt[:, jb, :],
                        lhsT=hsv[:, pair * 2 + jb, :],
                        rhs=wvres_sb[:, :],
                        start=True, stop=True)
                for jb in range(2):
                    bn = pair * 2 + jb
                    dst = vw_all[:, bn, :, :, 0:64]
                    src = vt[:, jb, :].rearrange("p (c j e) -> p c j e", c=2,
                                                 j=2)
                    if pair % 2 == 0:
                        nc.scalar.activation(dst, src, Copy)
                    else:
                        nc.vector.tensor_copy(dst, src)

            # ---- vw projection, first half: covers the weight DMA head ----
            with tc.tile_pool(name="vps1", bufs=4, space="PSUM") as vps1:
                for pair in range(NB // 4):
                    emit_vproj(pair, vps1)

            # ---- interleaved Wq/Wk projection streams ----
            with tc.tile_pool(name="wtq", bufs=3) as w_pool_q, \
                 tc.tile_pool(name="wtk", bufs=3) as w_pool_k, \
                 tc.tile_pool(name="stages", bufs=2) as st_pool, \
                 tc.tile_pool(name="vps2", bufs=2, space="PSUM") as vps2, \
                 tc.tile_pool(name="pp", bufs=4, space="PSUM") as pp_pool:

                stage_q = st_pool.tile([128, BC, NH, 128], f16, name="sq",
                                       tag="st")
                stage_k = st_pool.tile([128, BC, NH, 128], f16, name="sk",
                                       tag="st")

                def issue_chunk(dma_eng, w_pool, w_d, tg):
                    wt = w_pool.tile([128, WCHUNK, KTILES, 128], f16,
                                     name="wt", tag="wt")
                    dma_eng.dma_start(
                        wt[:, :, :, :],
                        w_d[:, tg * WCHUNK:(tg + 1) * WCHUNK, :]
                        .rearrange("p t (kt c) -> p t kt c", c=128))
                    return wt

                def emit_group(wt, dest, stage, tg):
                    pp = pp_pool.tile([128, WCHUNK, ROWS], f32, name="pp",
                                      tag="pp")
                    for ti in range(WCHUNK):
                        for kt in range(KTILES):
                            nc.tensor.matmul(
                                pp[:, ti, :],
                                lhsT=wt[:, ti, kt, :],
                                rhs=hsT[:, kt, :],
                                start=(kt == 0),
                                stop=(kt == KTILES - 1))
                    src = pp.rearrange(
                        "p ti (b n sp) -> p (b n) ti sp", n=NH, sp=4)
                    t0 = tg * WCHUNK
                    dv = dest[:, 0, :, :, :].rearrange(
                        "p b n (f sp) -> p (b n) f sp", sp=4)
                    sv = stage[:, :, :, :].rearrange(
                        "p b n (f sp) -> p (b n) f sp", sp=4)
                    nc.scalar.activation(
                        dv[:, :, t0:t0 + WCHUNK, :], src[0:64], Copy)
                    nc.vector.tensor_copy(
                        sv[64:128, :, t0:t0 + WCHUNK, :], src[64:128])

                # deep prefetch on both streams so transient HBM jitter can
                # be recovered instead of accumulating into a tail stall
                wq_tiles = [issue_chunk(nc.sync, w_pool_q, wq_d, t)
                            for t in range(2)]
                wk_tiles = [issue_chunk(nc.scalar, w_pool_k, wk_d, t)
                            for t in range(3)]
                vp = NB // 4
                for tg in range(NCHUNK):
                    if tg + 2 < NCHUNK:
                        wq_tiles.append(
                            issue_chunk(nc.sync, w_pool_q, wq_d, tg + 2))
                    emit_group(wq_tiles[tg], qt, stage_q, tg)
                    if tg + 3 < NCHUNK:
                        wk_tiles.append(
                            issue_chunk(nc.scalar, w_pool_k, wk_d, tg + 3))
                    emit_group(wk_tiles[tg], kt_, stage_k, tg)
                    if tg == NCHUNK - 1:
                        # qt partition shift 64..127 -> 0..63 (16KB runs)
                        nc.gpsimd.dma_start(
                            qt[:, 1, :, :, :], stage_q[64:128, :, :, :])
                    if tg >= NCHUNK - 4:
                        # vw-projection pairs as PE filler through the
                        # stream tail: late-chunk stalls hit these instead
                        # of idling the PE (which would latch k=4 clock)
                        for _ in range(3):
                            emit_vproj(vp, vps2)
                            vp += 1

                # kt partition shift, split across two idle queues
                h = BC // 2
                nc.gpsimd.dma_start(
                    kt_[:, 1, 0:h, :, :], stage_k[64:128, 0:h, :, :])
                nc.sync.dma_start(
                    kt_[:, 1, h:BC, :, :], stage_k[64:128, h:BC, :, :])

                # remaining vw pairs + a dep-free warm burst cover the shift
                while vp < NB // 2:
                    emit_vproj(vp, vps2)
                    vp += 1
                with tc.tile_pool(name="pwarm2", bufs=1,
                                  space="PSUM") as pw_pool2:
                    pw2 = pw_pool2.tile([A, ROWS], f32)
                    for wi in range(10):
                        nc.tensor.matmul(
                            pw2[:, :], lhsT=ones_bf[:, :], rhs=hsT[:, 0, :],
                            start=(wi == 0), stop=(wi == 9))

            # ---- attention: z^T matmuls + exp + folded AVW passes ----
            with tc.tile_pool(name="zps", bufs=2, space="PSUM") as z_pool, \
                 tc.tile_pool(name="aps", bufs=2, space="PSUM") as a_pool, \
                 tc.tile_pool(name="expz", bufs=2) as e_pool, \
                 tc.tile_pool(name="recs", bufs=2) as rec_pool, \
                 tc.tile_pool(name="tsum", bufs=2) as ts_pool, \
                 tc.tile_pool(name="fo", bufs=2) as f_pool:

                def emit_zt_exp(b):
                    zt = z_pool.tile([128, 4, 256], f32, name="zt", tag="zt")
                    for nh in range(NH):
                        for h in range(2):
                            nc.tensor.matmul(
                                zt[:, nh * 2 + h, :],
                                lhsT=kt_[:, h, b, nh, :],
                                rhs=qt[:, :, b, nh, :],
                                start=True, stop=True)
                    ez = e_pool.tile([128, 4, 256], bf16, name="ez", tag="ez")
                    nc.scalar.activation(
                        ez[:, :, :].rearrange("p a b -> p (a b)"),
                        zt[:, :, :].rearrange("p a b -> p (a b)"), Exp)
                    return ez

                fo = None
                ezs = emit_zt_exp(0)
                for b in range(BC):
                    cur = ezs
                    if b + 1 < BC:
                        ezs = emit_zt_exp(b + 1)
                    # 8 avw passes: queries (parity j) on M, [vw|1] on N
                    avw = a_pool.tile([128, 2, 2, 65], f32, name="avw",
                                      tag="avw")
                    for g in range(NH):
                        for j in range(2):
                            for kk in range(2):
                                nc.tensor.matmul(
                                    avw[:, g, j, :],
                                    lhsT=cur[:, g * 2 + kk,
                                             j * 128:(j + 1) * 128],
                                    rhs=vw_all[:, b * NH + g, kk, j, :],
                                    start=(kk == 0), stop=(kk == 1))
                    rec = rec_pool.tile([128, 4, 1], f32, name="rec",
                                        tag="rec")
                    nc.vector.reciprocal_approx_fast(
                        rec[:, :, :],
                        avw[:, :, :, 64:65].rearrange("p a b c -> p (a b) c"))
                    # per g: (slab_j0 * rec_j0) + (slab_j1 * rec_j1); bias
                    # already folded into the slabs (hsv ones row)
                    tsum = ts_pool.tile([128, 2, 64], f32, name="tsum",
                                        tag="tsum")
                    if b % 2 == 0:
                        fo = f_pool.tile([128, 2, 2, 64], f16, name="fo",
                                         tag="fo")
                    for g in range(NH):
                        nc.vector.tensor_scalar_mul(
                            tsum[:, g, :], avw[:, g, 0, 0:64],
                            rec[:, g * 2 + 0, :])
                        nc.vector.scalar_tensor_tensor(
                            tsum[:, g, :], avw[:, g, 1, 0:64],
                            rec[:, g * 2 + 1, :], tsum[:, g, :],
                            Mult, Add)
                    # relu + fp16 cast on Pool, then output DMA per pair
                    nc.gpsimd.tensor_scalar(
                        fo[:, b % 2, :, :].rearrange("p a b -> p (a b)"),
                        tsum[:, :, :].rearrange("p a b -> p (a b)"),
                        0.0, None, Max)
                    if b % 2 == 1:
                        nc.gpsimd.dma_start(
                            out_d[:, (b - 1) * 128:(b + 1) * 128],
                            fo[:, :, :, :].rearrange("p a g e -> p (a g e)"))
    nc.compile()
    return nc


def _get_nc():
    global _NC_CACHE
    if _NC_CACHE is None:
        _NC_CACHE = build_bass()
    return _NC_CACHE


def _prep_weight(W):
    # (CD, ND) -> (128, TTILES, KTILES*128): [p, t, kt*128+j] = W[kt*128+p, t*128+j]
    return np.ascontiguousarray(
        W.astype(np.float16).reshape(KTILES, 128, TTILES, 128)
        .transpose(1, 2, 0, 3).reshape(128, TTILES, KTILES * 128))


def make_in_maps(Hs, Wq, Wk, Wv, Wres_w, Wres_b):
    wq16 = _prep_weight(Wq)
    wk16 = _prep_weight(Wk)
    # folded output weights: wvres[E', chi, j, e] = sum_i Wv[E', chi*64+i]
    #   * Wres[j*64+i, e];  row E (ones-driven) carries bias/2 per j-slab
    wv = Wv.astype(np.float32).reshape(E, 2, A)            # (E', chi, i)
    wr = Wres_w.astype(np.float32).reshape(2, A, E)        # (j, i, e)
    wvres = np.einsum('xci,jie->xcje', wv, wr)             # (E', 2, 2, E)
    wvres_aug = np.concatenate(
        [wvres, np.broadcast_to(Wres_b.astype(np.float32) * 0.5,
                                (1, 2, 2, E))], axis=0)
    wvres16 = np.ascontiguousarray(
        wvres_aug.reshape(E + 1, 256).astype(np.float16))
    hs16 = Hs.astype(np.float16)
    maps = []
    for c in range(NCORES):
        sh = hs16[c * BC:(c + 1) * BC]                      # (BC, S, CD)
        hs2d = sh.reshape(ROWS, CD)
        hst = np.ascontiguousarray(
            hs2d.reshape(ROWS, KTILES, 128).transpose(2, 1, 0))
        # v rows in sigma order (f*4+sp):
        # hsv[e, q, f*4+sp] = Hs[b, nh*4+sp, f, e]; bn = 2q+pi = b*NH+nh
        arr = sh.reshape(NB, 4, F, E).transpose(0, 2, 1, 3).reshape(NB, 128, E)
        hsv = np.ascontiguousarray(arr.transpose(2, 0, 1))  # (E, NB, sigma)
        maps.append({
            "hst": hst, "hsv": hsv,
            "wq": wq16, "wk": wk16, "wvres": wvres16,
        })
    return maps


def _unpack_out(o):
    # o: (128, BC*128) rows q=(t, sp), cols (b, g, e) -> (BC, S, F*E)
    o = o.reshape(F, 4, BC, 2, E)                  # (t, sp, b, g, e)
    return np.ascontiguousarray(
        o.transpose(2, 3, 1, 0, 4)).reshape(BC, S, F * E)


def kernel(Hs, Wq, Wk, Wv, Wres_w, Wres_b):
    from concourse.bass_utils import run_bass_kernel_spmd
    nc = _get_nc()
    in_maps = make_in_maps(Hs, Wq, Wk, Wv, Wres_w, Wres_b)
    res = run_bass_kernel_spmd(nc, in_maps, list(range(NCORES)))
    out = np.concatenate(
        [_unpack_out(np.asarray(res.results[c]["out"]))
         for c in range(NCORES)], axis=0)
    return out.astype(np.float32)


if __name__ == "__main__":
    nc = build_bass()
    print("built OK; instructions:",
          sum(len(bb.instructions) for fn in nc.m.functions
              for bb in fn.blocks))
